# revision 1
# baseline (speedup 1.0000x reference)
"""Trainium2 Bass kernel for multi-head attention (B=2, P=2048, M=1024, N=16, H=64).

out = softmax(mask(x@Wq @ (x@Wk)^T / sqrt(H))) @ (x@Wv) @ Wo + biases,
with the module's strictly-upper-triangular keep mask (row P-1 fully masked).

Sharding: 8 cores = 2 batches x 4 head-groups. Core c handles batch c//4,
heads [4*(c%4), 4*(c%4)+4). Each core computes its heads' attention and the
partial output projection; the host sums partials across the 4 cores of each
batch. Measured: ~245 us HW exec, max rel err ~2.6e-3, resid_var ~1.8e-5
(bf16 matmuls, fp32 accumulate/softmax denominators).

Device algorithm (per core; bf16 matmuls, fp32 PSUM accumulation):
  - q^T,k^T,v^T [h', p] via projections with x^T (host-transposed) as the
    moving operand, head-pairs concatenated to fill 128 partitions; bias via
    K=1 matmul. One LDWEIGHTS feeds 4 accumulating p-tiles.
  - scores^T [pk, pq] with the triangular mask exploited by skipping
    fully-masked pk-chunks and narrowing partial ones. The two heads of a
    pair run CONCURRENTLY in disjoint PE row-groups (K=64 each) into one
    [128,1024] PSUM tile, so a single ScalarE exp covers both heads.
  - mask applied multiplicatively after exp (one [128,128] triangular
    constant); no -1e10 additive mask needed anywhere.
  - v is PE-transposed head-wise with an appended ones row, so the z matmul
    (z_aug^T = v_aug^T @ exp^T) also produces the softmax denominators.
  - z matmuls trail the scores/exp stream by a sliding window of DW slots so
    the in-order PE stream never blocks on ScalarE exp latency.
  - z_aug^T is PE-transposed to [pq, h] where the denominator is a
    per-partition scalar: reciprocal + tensor_scalar normalize, transpose
    back, head-pairs packed to K=128 for the output projection which
    accumulates both pairs in one PSUM bank.
  - The fully-masked query row P-1 (softmax of all -1e10 = uniform
    attention) is patched analytically on the host:
    out[b,P-1] = sum_n (mean_p x[b] @ Wv[n] + bv[n]) @ Wo[n] + bias_out.
  - Engine balance: exp + q/k/v^T PSUM evacuation on ScalarE, masks/
    normalize/output copies on VectorE, ones-rows on GpSimd, DMA split
    across the SP and ACT hardware queues.

KERNEL_DT=f32r selects an experimental fp32r build (~1.6e-4 err, slower);
the graded default is bf16.
"""
import os
import sys

import numpy as np

if "/opt/trn_rl_repo" not in sys.path:
    sys.path.insert(0, "/opt/trn_rl_repo")

import concourse.bacc as bacc
import concourse.tile as tile
from concourse import mybir
from concourse import bass_utils
import ml_dtypes

B, P, M, N, H = 2, 2048, 1024, 16, 64
NCORES = 8
HPC = 4          # heads per core
NPAIRS = 2       # head pairs per core
MK = M // 128    # 8 contraction chunks for projections
PT = P // 512    # 4 free-dim tiles of 512 over sequence
PC = P // 128    # 16 partition chunks over sequence
MT = M // 512    # 2 output m-tiles

F32 = mybir.dt.float32
F32R = mybir.dt.float32r
BF16 = mybir.dt.bfloat16
EXP = mybir.ActivationFunctionType.Exp
MULT = mybir.AluOpType.mult

DT_MODE = os.environ.get("KERNEL_DT", "bf16")   # "bf16" | "f32r"
DT_MM = BF16 if DT_MODE == "bf16" else F32R
NP_MM = ml_dtypes.bfloat16 if DT_MODE == "bf16" else np.float32

_BUILT = {}


def _emit(nc, tc, aps, ctx):
    xT = aps["xT"]          # [1024, 2048]
    outp = aps["outp"]      # [2048, 1024]

    consts = ctx.enter_context(tc.tile_pool(name="consts", bufs=1))
    xpool = ctx.enter_context(tc.tile_pool(name="xpool", bufs=MK))
    qkpool = ctx.enter_context(tc.tile_pool(name="qkpool", bufs=2))
    vapool = ctx.enter_context(tc.tile_pool(name="vapool", bufs=68))
    zppool = ctx.enter_context(tc.tile_pool(name="zppool", bufs=16))
    expool = ctx.enter_context(
        tc.tile_pool(name="expool", bufs=(9 if DT_MODE == "bf16" else 9)))
    zsbpool = ctx.enter_context(tc.tile_pool(name="zsbpool", bufs=6))
    znpool = ctx.enter_context(tc.tile_pool(name="znpool", bufs=6))
    rcpool = ctx.enter_context(tc.tile_pool(name="rcpool", bufs=8))
    opool = ctx.enter_context(tc.tile_pool(name="opool", bufs=4))

    eye = consts.tile([128, 128], F32)
    nc.sync.dma_start(eye[:], aps["eye"][:])
    mask = consts.tile([128, 128], DT_MM)
    nc.sync.dma_start(mask[:], aps["mask"][:])
    ones32 = consts.tile([1, 512], F32)
    nc.vector.memset(ones32[:], 1.0)
    if DT_MODE == "bf16":
        ones_mm = consts.tile([1, 512], BF16)
        nc.vector.memset(ones_mm[:], 1.0)
    else:
        ones_mm = consts.tile([1, 512], F32R)
        nc.vector.tensor_copy(ones_mm[:], ones32[:])
    wos = []
    for pr in range(NPAIRS):
        wot = consts.tile([128, 1024], DT_MM, tag=f"wo{pr}", name=f"wo{pr}")
        nc.sync.dma_start(wot[:], aps["wo"][pr])
        wos.append(wot)

    # x^T chunks [128 m, 2048 p]
    xsb = []
    for k in range(MK):
        xt = xpool.tile([128, 2048], DT_MM, tag="x")
        # split each chunk across both HWDGE queues so the first
        # projection group isn't gated by serialized 512KB transfers
        nc.sync.dma_start(xt[:, 0:1024], xT[128 * k:128 * (k + 1), 0:1024])
        nc.scalar.dma_start(
            xt[:, 1024:2048], xT[128 * k:128 * (k + 1), 1024:2048]
        )
        xsb.append(xt)

    tiles = {}
    qts, kts = {}, {}
    vts_set = []

    def finish_pair(pr, j, zpss, t_pool):
        """Copy both heads' z_aug^T out of PSUM, then normalize in pq-space
        with the two heads' transpose chains interleaved (hides the
        PE->DVE->PE latency of each chain)."""
        zsbs = []
        for h01 in range(2):
            zsb = zsbpool.tile([65, 512], F32, tag="z",
                               name=f"zsb{pr}_{h01}_{j}")
            nc.vector.tensor_copy(zsb[:], zpss[h01][:])
            if j == PT - 1:
                # fully-masked query row P-1: denom 0 -> 1 so the reciprocal
                # is finite (host patches the output row)
                nc.vector.tensor_copy(zsb[64:65, 511:512], ones32[:, 0:1])
            zsbs.append(zsb)
        for c4 in range(4):
            psts = []
            for h01 in range(2):
                pst1 = t_pool.tile([128, 65], F32, tag="tps", bufs=2,
                                   name=f"pst1_{pr}_{h01}_{j}_{c4}")
                nc.tensor.transpose(
                    pst1[:], zsbs[h01][:, 128 * c4:128 * (c4 + 1)],
                    eye[0:65, 0:65],
                )
                psts.append(pst1)
            zns = []
            for h01 in range(2):
                rcol = rcpool.tile([128, 1], F32, tag="rc")
                nc.vector.reciprocal(rcol[:], psts[h01][:, 64:65])
                zn = znpool.tile([128, 64], F32, tag="zn")
                nc.vector.tensor_scalar_mul(zn[:], psts[h01][:, 0:64],
                                            rcol[:])
                zns.append(zn)
            for h01 in range(2):
                rows = slice(64 * h01, 64 * (h01 + 1))
                pst2 = t_pool.tile([64, 128], F32, tag="tps", bufs=2,
                                   name=f"pst2_{pr}_{h01}_{j}_{c4}")
                nc.tensor.transpose(pst2[:], zns[h01][:], eye[:])
                nc.vector.tensor_copy(
                    tiles[("zp", pr, 4 * j + c4)][rows, :], pst2[:]
                )

    def proj(j, ps_pool):
        for c4 in range(4):
            ck = 4 * j + c4
            for mt in range(MT):
                pp = ps_pool.tile([128, 512], F32, tag="tps", bufs=2,
                                  name=f"prps{ck}_{mt}")
                nc.tensor.matmul(
                    pp[:], tiles[("zp", 0, ck)][:],
                    wos[0][:, 512 * mt:512 * (mt + 1)],
                    start=True, stop=False,
                )
                nc.tensor.matmul(
                    pp[:], tiles[("zp", 1, ck)][:],
                    wos[1][:, 512 * mt:512 * (mt + 1)],
                    start=False, stop=True,
                )
                osb = opool.tile([128, 512], F32, tag="osb")
                nc.vector.tensor_copy(osb[:], pp[:])
                nc.sync.dma_start(
                    outp[128 * ck:128 * (ck + 1), 512 * mt:512 * (mt + 1)],
                    osb[:],
                )

    def attn_small(pr, j, ps_pool):
        """Single-chunk [128,512] attention for short j (few kept chunks);
        round-robin over the pair's two heads, z trailing by DW slots."""
        qT, kT = qts[pr], kts[pr]
        ilist = list(range(PC - 1, 4 * j - 1, -1))
        nchunk = len(ilist)
        nslot = 2 * nchunk
        DW = min(4, nslot - 1)
        zpss = [ps_pool.tile([65, 512], F32, tag="qkvps",
                             name=f"zpss{pr}_{h01}_{j}")
                for h01 in range(2)]
        descs = []
        for idx in range(nslot + DW):
            if idx < nslot:
                h01, a = idx % 2, idx // 2
                rows = slice(64 * h01, 64 * (h01 + 1))
                i_ = ilist[a]
                tt = i_ - 4 * j
                w_ = min(512, 128 * (tt + 1))
                sps = ps_pool.tile([128, 512], F32, tag="qkvps",
                                   name=f"ssps{pr}_{h01}_{j}_{a}")
                nc.tensor.matmul(
                    sps[:, :w_],
                    kT[rows, 128 * i_:128 * (i_ + 1)],
                    qT[rows, 512 * j:512 * j + w_],
                    start=True, stop=True,
                )
                ex = expool.tile([128, 1024], DT_MM, tag="ex")
                nc.scalar.activation(ex[:, :w_], sps[:, :w_], EXP,
                                     scale=0.125)
                if tt < 4:
                    nc.vector.tensor_mul(
                        ex[:, 128 * tt:w_], ex[:, 128 * tt:w_], mask[:]
                    )
                descs.append((ex, h01, i_, w_))
            zi = idx - DW
            if 0 <= zi < nslot:
                ex, h01, i_, w_ = descs[zi]
                nc.tensor.matmul(
                    zpss[h01][:, :w_], tiles[("va", pr, h01, i_)][:],
                    ex[:, :w_],
                    start=(zi < 2), stop=(zi >= nslot - 2),
                )
        finish_pair(pr, j, zpss, ps_pool)

    def attn_big(pr, j, sc_pool, z_pool, t_pool):
        """Row-packed attention: both heads' K=64 score matmuls run
        concurrently in disjoint PE row-groups into one [128,1024] PSUM
        tile; one batched exp covers both. z matmuls trail by DW slots."""
        qT, kT = qts[pr], kts[pr]
        ilist = list(range(PC - 1, 4 * j - 1, -1))
        nslot = len(ilist)
        DW = min(6, nslot - 1)
        zpss = [z_pool.tile([65, 512], F32, tag="zps",
                            name=f"zps{pr}_{h01}_{j}")
                for h01 in range(2)]
        descs = []
        for idx in range(nslot + DW):
            if idx < nslot:
                i_ = ilist[idx]
                tt = i_ - 4 * j
                w_ = min(512, 128 * (tt + 1))
                sps = sc_pool.tile([128, 1024], F32, tag="scps")
                nc.tensor.matmul(
                    sps[:, :w_],
                    kT[0:64, 128 * i_:128 * (i_ + 1)],
                    qT[0:64, 512 * j:512 * j + w_],
                    start=True, stop=True,
                )
                nc.tensor.matmul(
                    sps[:, 512:512 + w_],
                    kT[64:128, 128 * i_:128 * (i_ + 1)],
                    qT[64:128, 512 * j:512 * j + w_],
                    start=True, stop=True,
                )
                ex = expool.tile([128, 1024], DT_MM, tag="ex")
                if w_ == 512:
                    nc.scalar.activation(ex[:], sps[:], EXP, scale=0.125)
                else:
                    nc.scalar.activation(ex[:, :w_], sps[:, :w_], EXP,
                                         scale=0.125)
                    nc.scalar.activation(
                        ex[:, 512:512 + w_], sps[:, 512:512 + w_], EXP,
                        scale=0.125,
                    )
                if tt < 4:
                    for off in (0, 512):
                        nc.vector.tensor_mul(
                            ex[:, off + 128 * tt:off + w_],
                            ex[:, off + 128 * tt:off + w_], mask[:]
                        )
                descs.append((ex, i_, w_))
            zi = idx - DW
            if 0 <= zi < nslot:
                ex, i_, w_ = descs[zi]
                nc.tensor.matmul(
                    zpss[0][:, :w_], tiles[("va", pr, 0, i_)][:],
                    ex[:, :w_],
                    start=(zi == 0), stop=(zi == nslot - 1),
                )
                nc.tensor.matmul(
                    zpss[1][:, :w_], tiles[("va", pr, 1, i_)][:],
                    ex[:, 512:512 + w_],
                    start=(zi == 0), stop=(zi == nslot - 1),
                )
        finish_pair(pr, j, zpss, t_pool)

    for pr in range(NPAIRS):
        for c4 in range(4):
            tiles[("zp", pr, 4 * (PT - 1) + c4)] = zppool.tile(
                [128, 128], DT_MM, tag="zp", name=f"zp{pr}_{4 * (PT - 1) + c4}")

    # ---- QKV projections, with the short j=3 attention interleaved ----
    with tc.tile_pool(name="wpool", bufs=6) as wpool, \
         tc.tile_pool(name="vtpool", bufs=4) as vtpool, \
         tc.tile_pool(name="ps_qkv", bufs=6, space="PSUM") as ps_qkv:
        wsb = {}
        bsb = {}
        for pr in range(NPAIRS):
            for t in ("v", "q", "k"):
                bt = consts.tile([1, 128], DT_MM, tag=f"b{t}{pr}")
                nc.scalar.dma_start(bt[:], aps[f"b{t}"][pr])
                bsb[(t, pr)] = bt
        for pr in range(NPAIRS):
            for t in ("v", "q", "k"):
                wt = wpool.tile([128, MK * 128], DT_MM, tag="w",
                                name=f"w_{t}{pr}")
                nc.scalar.dma_start(
                    wt.rearrange("p (k f) -> p k f", k=MK),
                    aps[f"w{t}"][pr].rearrange("k p f -> p k f"),
                )
                wsb[(t, pr)] = wt
        for pr in range(NPAIRS):
            qT = qkpool.tile([128, 2048], DT_MM, tag="qT", name=f"qT{pr}")
            kT = qkpool.tile([128, 2048], DT_MM, tag="kT", name=f"kT{pr}")
            qts[pr], kts[pr] = qT, kT
            # v first, using only 2 PSUM slots so the q/k projections can
            # overlap the DVE-paced v-transpose section
            for j4a in range(0, PT, 2):
                w = wsb[("v", pr)]
                pss = [ps_qkv.tile([128, 512], F32, tag="qkvps",
                                   name=f"qkvps_v{pr}{j4a + d}")
                       for d in range(2)]
                for mk in range(MK):
                    for d in range(2):
                        nc.tensor.matmul(
                            pss[d][:],
                            w[:, 128 * mk:128 * (mk + 1)],
                            xsb[mk][:, 512 * (j4a + d):512 * (j4a + d + 1)],
                            start=(mk == 0), stop=False,
                        )
                for d in range(2):
                    nc.tensor.matmul(
                        pss[d][:], bsb[("v", pr)][:],
                        ones_mm[:], start=False, stop=True,
                    )
                for d in range(2):
                    j4 = j4a + d
                    ps = pss[d]
                    # v^T slice + ones row, PE-transposed into v_aug
                    # chunks [128 pk, 65] (col 64 = ones for denoms).
                    # vts tiles persist; their ones row is written once.
                    for h01 in range(2):
                        if len(vts_set) < 4:
                            vts = vtpool.tile([65, 512], F32, tag="vT",
                                              name=f"vts{len(vts_set)}")
                            nc.gpsimd.tensor_copy(vts[64:65, :], ones32[:])
                            vts_set.append(vts)
                        vts = vts_set[(2 * d + h01) % 4]
                        nc.scalar.copy(
                            vts[0:64, :], ps[64 * h01:64 * (h01 + 1), :]
                        )
                        for c4 in range(4):
                            pst = ps_qkv.tile([128, 65], F32, tag="qkvps")
                            nc.tensor.transpose(
                                pst[:], vts[:, 128 * c4:128 * (c4 + 1)],
                                eye[0:65, 0:65],
                            )
                            va = vapool.tile([128, 65], DT_MM, tag="va")
                            nc.vector.tensor_copy(va[:], pst[:])
                            tiles[("va", pr, h01, 4 * j4 + c4)] = va
            for t, dest in (("q", qT), ("k", kT)):
                w = wsb[(t, pr)]
                # one LDWEIGHTS per m-chunk feeds 4 accumulating p-tiles
                pss = [ps_qkv.tile([128, 512], F32, tag="qkvps",
                                   name=f"qkvps_{t}{pr}{j4}")
                       for j4 in range(PT)]
                for mk in range(MK):
                    for j4 in range(PT):
                        nc.tensor.matmul(
                            pss[j4][:],
                            w[:, 128 * mk:128 * (mk + 1)],
                            xsb[mk][:, 512 * j4:512 * (j4 + 1)],
                            start=(mk == 0), stop=False,
                        )
                for j4 in range(PT):
                    nc.tensor.matmul(
                        pss[j4][:], bsb[(t, pr)][:],
                        ones_mm[:], start=False, stop=True,
                    )
                for j4 in range(PT):
                    nc.scalar.copy(
                        dest[:, 512 * j4:512 * (j4 + 1)], pss[j4][:]
                    )
            # short j=PT-1 attention for this pair, hidden in the qkv stream
            attn_small(pr, PT - 1, ps_qkv)
        proj(PT - 1, ps_qkv)

    # ---- deep-pipelined attention for the remaining j ----
    with tc.tile_pool(name="ps_sc", bufs=2, space="PSUM") as ps_sc, \
         tc.tile_pool(name="ps_z", bufs=2, space="PSUM") as ps_z, \
         tc.tile_pool(name="ps_t", bufs=2, space="PSUM") as ps_t:
        for j in range(PT - 2, -1, -1):
            for pr in range(NPAIRS):
                for c4 in range(4):
                    tiles[("zp", pr, 4 * j + c4)] = zppool.tile(
                        [128, 128], DT_MM, tag="zp",
                        name=f"zp{pr}_{4 * j + c4}")
            for pr in range(NPAIRS):
                attn_big(pr, j, ps_sc, ps_z, ps_t)
            proj(j, ps_t)


def _build():
    if DT_MODE in _BUILT:
        return _BUILT[DT_MODE]
    from contextlib import ExitStack

    nc = bacc.Bacc("TRN2", target_bir_lowering=False, debug=False)
    aps = {
        "xT": nc.dram_tensor("xT", [M, P], DT_MM, kind="ExternalInput").ap(),
        "wq": nc.dram_tensor("wq", [NPAIRS, MK, 128, 128], DT_MM,
                             kind="ExternalInput").ap(),
        "wk": nc.dram_tensor("wk", [NPAIRS, MK, 128, 128], DT_MM,
                             kind="ExternalInput").ap(),
        "wv": nc.dram_tensor("wv", [NPAIRS, MK, 128, 128], DT_MM,
                             kind="ExternalInput").ap(),
        "wo": nc.dram_tensor("wo", [NPAIRS, 128, 1024], DT_MM,
                             kind="ExternalInput").ap(),
        "bq": nc.dram_tensor("bq", [NPAIRS, 1, 128], DT_MM,
                             kind="ExternalInput").ap(),
        "bk": nc.dram_tensor("bk", [NPAIRS, 1, 128], DT_MM,
                             kind="ExternalInput").ap(),
        "bv": nc.dram_tensor("bv", [NPAIRS, 1, 128], DT_MM,
                             kind="ExternalInput").ap(),
        "eye": nc.dram_tensor("eye", [128, 128], F32,
                              kind="ExternalInput").ap(),
        "mask": nc.dram_tensor("mask", [128, 128], DT_MM,
                               kind="ExternalInput").ap(),
        "outp": nc.dram_tensor("outp", [P, M], F32, kind="ExternalOutput").ap(),
    }
    with tile.TileContext(nc) as tc:
        with ExitStack() as ctx:
            _emit(nc, tc, aps, ctx)
    nc.compile()
    _BUILT[DT_MODE] = nc
    return nc


def _host_inputs(x, kq, kk, kv, ko, bq, bk, bv):
    xT = np.ascontiguousarray(x.transpose(0, 2, 1)).astype(NP_MM)  # [B, M, P]
    eye = np.eye(128, dtype=np.float32)
    # keep iff pq < pk; block mask[r(pk), c(pq)] = 1 if c < r
    mask = np.tril(np.ones((128, 128), np.float32), k=-1).astype(NP_MM)
    in_maps = []
    for c in range(NCORES):
        b, k4 = divmod(c, 4)
        heads = [4 * k4 + i for i in range(HPC)]

        def pairw(kern):
            # [NPAIRS, MK, 128, 128] lhsT chunks
            out = np.empty((NPAIRS, MK, 128, 128), NP_MM)
            for pr in range(NPAIRS):
                pairm = np.concatenate(
                    [kern[heads[2 * pr]], kern[heads[2 * pr + 1]]], axis=1
                )  # [1024, 128]
                out[pr] = pairm.reshape(MK, 128, 128).astype(NP_MM)
            return out

        def pairb(bias):
            out = np.empty((NPAIRS, 1, 128), NP_MM)
            for pr in range(NPAIRS):
                out[pr, 0] = np.concatenate(
                    [bias[heads[2 * pr]], bias[heads[2 * pr + 1]]]
                ).astype(NP_MM)
            return out

        wo = np.empty((NPAIRS, 128, 1024), NP_MM)
        for pr in range(NPAIRS):
            wo[pr] = np.concatenate(
                [ko[heads[2 * pr]], ko[heads[2 * pr + 1]]], axis=0
            ).astype(NP_MM)

        in_maps.append({
            "xT": xT[b],
            "wq": pairw(kq), "wk": pairw(kk), "wv": pairw(kv),
            "wo": wo,
            "bq": pairb(bq), "bk": pairb(bk), "bv": pairb(bv),
            "eye": eye, "mask": mask,
        })
    return in_maps


def kernel(x, kernel_query, kernel_key, kernel_value, kernel_out,
           bias_query, bias_key, bias_value, bias_out, _trace=False):
    x = np.asarray(x, np.float32)
    kq = np.asarray(kernel_query, np.float32)
    kk = np.asarray(kernel_key, np.float32)
    kv = np.asarray(kernel_value, np.float32)
    ko = np.asarray(kernel_out, np.float32)
    bq = np.asarray(bias_query, np.float32)
    bk = np.asarray(bias_key, np.float32)
    bv = np.asarray(bias_value, np.float32)
    bo = np.asarray(bias_out, np.float32)

    nc = _build()
    in_maps = _host_inputs(x, kq, kk, kv, ko, bq, bk, bv)
    res = bass_utils.run_bass_kernel_spmd(
        nc, in_maps, core_ids=list(range(NCORES)), trace=_trace
    )
    out = np.zeros((B, P, M), np.float32)
    for c in range(NCORES):
        out[c // 4] += res.results[c]["outp"]
    out += bo[None, None, :]

    # patch fully-masked query row P-1: uniform attention = mean_k v
    for b in range(B):
        xbar = x[b].mean(axis=0, dtype=np.float64)  # [M]
        row = np.zeros(M, np.float64)
        for n in range(N):
            zrow = xbar @ kv[n].astype(np.float64) + bv[n].astype(np.float64)
            row += zrow @ ko[n].astype(np.float64)
        out[b, P - 1, :] = (row + bo.astype(np.float64)).astype(np.float32)

    if _trace:
        kernel._last_result = res
    return out



# revision 16
# speedup vs baseline: 1.2166x; 1.2166x over previous
"""Trainium2 Bass kernel for multi-head attention (B=2, P=2048, M=1024, N=16, H=64).

out = softmax(mask(x@Wq @ (x@Wk)^T / sqrt(H))) @ (x@Wv) @ Wo + biases,
with the module's strictly-upper-triangular keep mask (row P-1 fully masked).

Sharding: 8 cores = 2 batches x 4 head-groups. Core c handles batch c//4,
heads [4*(c%4), 4*(c%4)+4); the host sums the 4 partial output projections
per batch and patches the fully-masked query row P-1 analytically.

v3 design (fp8 DoubleRow where precision allows, ACT-exp-bound):
  - q/k projections (K=1024) run as fp8e4 DoubleRow matmuls (2 K-chunks
    per pass, 0.5 cyc/row). The v projection runs in bf16: fp8 v errors
    hit concentrated-attention rows at full strength (measured).
  - z = v_aug @ exp keeps DoubleRow speed at 16-bit-grade v precision:
    v_aug^T is stored as an fp8 hi+lo residual pair (v = hi + lo + O(e^2))
    in the two DR planes, and the fp8 ex operand is duplicated across
    planes with a stride-0 AP. Output projection runs in fp16.
  - Scores stay bf16, two heads row-packed in disjoint PE row groups.
  - The triangular mask is applied ADDITIVELY on the PE (eye @ maskneg
    accumulated into score PSUM): exp then yields exact fp8 zeros, so no
    DVE masking and no garbage in the DoubleRow-widened columns.
  - Softmax denominators come from an appended ones column in v_aug^T;
    normalization = DVE reciprocal of the denom row -> gpsimd
    partition_broadcast -> one DVE multiply straight into the fp8 zp tile
    (replaces the baseline's 4 PE transposes per chunk).
  - ACT runs exp only (one activation per pk chunk covers both heads);
    all PSUM evacuation is on DVE; DMA issue on the sync queue.
  - x DMA is chunked/ordered so the first projection starts ~1us in.
  - Timeline: proj(pr0) -> attention(pr0, j=3..0) with proj(pr1)+v1
    transposes interleaved into PE slack -> attention(pr1) with the
    output projection per j trailing.
"""
import sys

import numpy as np

if "/opt/trn_rl_repo" not in sys.path:
    sys.path.insert(0, "/opt/trn_rl_repo")

import concourse.bacc as bacc
import concourse.tile as tile
from concourse import mybir
from concourse import bass_utils
import ml_dtypes

B, P, M, N, H = 2, 2048, 1024, 16, 64
NCORES = 8
HPC = 4          # heads per core
NPAIRS = 2       # head pairs per core
MKD = 4          # DoubleRow contraction chunks (256 each) for projections
PT = P // 512    # 4 pq tiles of 512
PC = P // 128    # 16 pk chunks of 128

F32 = mybir.dt.float32
F16 = mybir.dt.float16
BF16 = mybir.dt.bfloat16
FP8 = mybir.dt.float8e4
NP_FP8 = ml_dtypes.float8_e4m3
NP_BF16 = ml_dtypes.bfloat16
EXP = mybir.ActivationFunctionType.Exp
DR = mybir.MatmulPerfMode.DoubleRow
MASKC = -240.0   # exp(0.125 * -240) = e^-30 -> exact fp8 zero

_BUILT = {}


def _emit(nc, tc, aps, ctx):
    outp = aps["outp"]      # [2048, 1024] f16

    consts = ctx.enter_context(tc.tile_pool(name="consts", bufs=1))
    xpool = ctx.enter_context(tc.tile_pool(name="xpool", bufs=8))
    wpool = ctx.enter_context(tc.tile_pool(name="wpool", bufs=7))
    qkpool = ctx.enter_context(tc.tile_pool(name="qkpool", bufs=4))
    vtpool = ctx.enter_context(tc.tile_pool(name="vtpool", bufs=4))
    vapool = ctx.enter_context(tc.tile_pool(name="vapool", bufs=4))
    expool = ctx.enter_context(tc.tile_pool(name="expool", bufs=4))
    zppool = ctx.enter_context(tc.tile_pool(name="zppool", bufs=4))
    rcpool = ctx.enter_context(tc.tile_pool(name="rcpool", bufs=4))
    bcpool = ctx.enter_context(tc.tile_pool(name="bcpool", bufs=4))
    opool = ctx.enter_context(tc.tile_pool(name="opool", bufs=3))

    # ---- constants ----
    eyebf = consts.tile([128, 128], BF16)
    nc.scalar.dma_start(eyebf[:], aps["eyebf"][:])
    maskneg = consts.tile([128, 256], BF16)
    nc.scalar.dma_start(maskneg[:], aps["maskneg"][:])
    bcol = consts.tile([128, 6], F32)   # (q0,k0,v0,q1,k1,v1)
    nc.scalar.dma_start(bcol[:], aps["bcol"][:])

    # ---- weights ----
    wsb = {}
    for pr in range(NPAIRS):
        for t in ("q", "k"):
            wt = wpool.tile([128, MKD, 2, 128], FP8, tag="w", name=f"w{t}{pr}")
            nc.scalar.dma_start(
                wt[:], aps[f"w{t}"][pr].rearrange("c p i m -> p c i m")
            )
            wsb[(t, pr)] = wt
        wt = wpool.tile([128, 8, 128], BF16, tag="wv", name=f"wv{pr}")
        nc.scalar.dma_start(
            wt[:], aps["wv"][pr].rearrange("c p m -> p c m")
        )
        wsb[("v", pr)] = wt
    wosb = wpool.tile([128, 2, 1024], F16, tag="w", name="wo")
    nc.scalar.dma_start(wosb[:], aps["wo"][:])

    # ---- x chunks: 8 fp8 tiles [128, 2, 1024] (mk-pair c x seq-half h)
    # for the q/k DR projections, plus 8 bf16 m-chunk tiles for v ----
    xsb = {}
    for h in range(2):
        for c in range(MKD):
            xt = xpool.tile([128, 2, 1024], FP8, tag="x", name=f"x{c}{h}")
            nc.sync.dma_start(
                xt[:], aps["x8"][c][:, :, 1024 * h:1024 * (h + 1)]
            )
            xsb[(c, h)] = xt
    xbf = {}
    for mk in range(8):
        xt = xpool.tile([128, 2048], BF16, tag="xb", name=f"xb{mk}")
        nc.scalar.dma_start(xt[:, 0:1024], aps["xbf"][mk][:, 0:1024])
        nc.sync.dma_start(xt[:, 1024:2048], aps["xbf"][mk][:, 1024:2048])
        xbf[mk] = xt

    qts, kts = {}, {}
    vas = {}
    BIDX = {"q": 0, "k": 1, "v": 2}

    # persistent vts staging tiles (ones row written once, reused)
    vts_tiles = []
    for i in range(4):
        vt = vtpool.tile([65, 512], BF16, tag="vt", name=f"vts{i}")
        nc.vector.memset(vt[64:65, :], 1.0)
        vts_tiles.append(vt)
    _vts_ctr = [0]

    for pr in range(NPAIRS):
        qt = qkpool.tile([128, 2048], F16, tag="qk", name=f"qT{pr}")
        kt = qkpool.tile([128, 2048], F16, tag="qk", name=f"kT{pr}")
        qts[pr], kts[pr] = qt, kt
        for h01 in range(2):
            va = vapool.tile([128, PC, 2, 80], FP8, tag="va",
                             name=f"va{pr}{h01}")
            vas[(pr, h01)] = va

    def proj_group(t, pr, h, ps_pool):
        """One projection group: DR-fp8 matmul of type t, pair pr, seq half
        h (pq columns [1024h, 1024h+1024)) into two [128,512] PSUM tiles;
        evacuate on DVE with the bias fold."""
        w = wsb[(t, pr)]
        pps = []
        for d in range(2):
            pp = ps_pool.tile([128, 512], F32, tag="aux",
                              name=f"prj_{t}{pr}{h}{d}")
            pps.append(pp)
        if t in ("q", "k"):
            for c in range(MKD):
                for d in range(2):
                    nc.tensor.matmul(
                        pps[d][:],
                        w[:, c, :, :],
                        xsb[(c, h)][:, :, 512 * d:512 * (d + 1)],
                        start=(c == 0), stop=(c == MKD - 1),
                        perf_mode=DR,
                    )
        else:
            for mk in range(8):
                for d in range(2):
                    col = 1024 * h + 512 * d
                    nc.tensor.matmul(
                        pps[d][:],
                        w[:, mk, :],
                        xbf[mk][:, col:col + 512],
                        start=(mk == 0), stop=(mk == 7),
                    )
        bias = bcol[:, BIDX[t] + 3 * pr:BIDX[t] + 3 * pr + 1]
        if t in ("q", "k"):
            dest = (qts if t == "q" else kts)[pr]
            for d in range(2):
                nc.vector.tensor_scalar_add(
                    dest[:, 1024 * h + 512 * d:1024 * h + 512 * (d + 1)],
                    pps[d][:], bias,
                )
        else:
            # v: evacuate all four (head, j4) slices first (frees the pps
            # for the ring), then PE-transpose each to [128 pk, 65] and
            # cast into the fp8 va planes
            units = []
            for d in range(2):
                j4 = 2 * h + d
                for h01 in range(2):
                    vt = vts_tiles[_vts_ctr[0] % 4]
                    _vts_ctr[0] += 1
                    nc.vector.tensor_scalar_add(
                        vt[0:64, :],
                        pps[d][64 * h01:64 * (h01 + 1), :],
                        bcol[64 * h01:64 * (h01 + 1),
                             BIDX[t] + 3 * pr:BIDX[t] + 3 * pr + 1],
                    )
                    units.append((vt, h01, j4))
            for vt, h01, j4 in units:
                pst = ps_pool.tile([128, 4, 66], BF16, tag="aux",
                                   name=f"pst{pr}{h01}{j4}")
                for c4 in range(4):
                    nc.tensor.transpose(
                        pst[:, c4, 0:65],
                        vt[:, 128 * c4:128 * (c4 + 1)],
                        eyebf[0:65, 0:65],
                    )
                vhi = vas[(pr, h01)][:, 4 * j4:4 * j4 + 4, 0, 0:65]
                nc.vector.tensor_copy(vhi, pst[:, :, 0:65])
                nc.vector.tensor_sub(
                    vas[(pr, h01)][:, 4 * j4:4 * j4 + 4, 1, 0:65],
                    pst[:, :, 0:65], vhi,
                )

    def attn_pair(pr, j, sps_pool, zps_pool):
        """Attention for head-pair pr, pq tile j: bf16 row-packed scores
        with PE-additive triangular mask, one exp per pk chunk (both
        heads), fp8 DoubleRow z accumulation over pk chunk pairs, then
        broadcast-normalize into the fp8 zp tile (plane pr)."""
        qt, kt = qts[pr], kts[pr]
        nchunk = PC - 4 * j
        DW = min(3, nchunk)
        zpss = [zps_pool.tile([65, 512], F32, tag="zps",
                              name=f"zps{pr}{j}{h01}")
                for h01 in range(2)]
        descs = []
        for idx in range(nchunk + DW):
            if idx < nchunk:
                i_ = PC - 1 - idx              # descending pk chunks
                tt = i_ - 4 * j
                wp = min(512, 128 * (tt + 1))
                ex = expool.tile([128, 1024], FP8, tag="ex")
                sps = sps_pool.tile([128, 1024], F32, tag="sc")
                for h01 in range(2):
                    rows = slice(64 * h01, 64 * (h01 + 1))
                    nc.tensor.matmul(
                        sps[:, 512 * h01:512 * h01 + wp],
                        kt[rows, 128 * i_:128 * (i_ + 1)],
                        qt[rows, 512 * j:512 * j + wp],
                        start=True, stop=(tt >= 4),
                    )
                if tt < 4:
                    for h01 in range(2):
                        off = 512 * h01 + 128 * tt
                        nc.tensor.matmul(
                            sps[:, off:off + 128],
                            eyebf[:],
                            maskneg[:, 0:128],
                            start=False, stop=True,
                            skip_group_check=True,
                        )
                # one exp covers both heads -> fp8 ex
                nc.scalar.activation(
                    ex[:].rearrange(
                        "p (two f) -> p two f", two=2)[:, :, 0:wp],
                    sps[:].rearrange(
                        "p (two f) -> p two f", two=2)[:, :, 0:wp],
                    EXP, scale=0.125,
                )
                descs.append((ex, i_, wp))
            zi = idx - DW
            if 0 <= zi < nchunk:
                ex, i_, wp = descs[zi]
                for h01 in range(2):
                    nc.tensor.matmul(
                        zpss[h01][:, 0:wp],
                        vas[(pr, h01)][:, i_, :, 0:65],
                        ex[:, 512 * h01:512 * h01 + wp]
                        .unsqueeze(1).broadcast_to((128, 2, wp)),
                        start=(zi == 0), stop=(zi == nchunk - 1),
                        perf_mode=DR,
                    )
        # normalize: recip(denom row) -> partition_broadcast -> multiply
        zpj = zp_tiles[j]
        for h01 in range(2):
            if j == PT - 1:
                # fully-masked query row P-1: denom 0 -> 1 (host patches)
                nc.vector.memset(zpss[h01][64:65, 511:512], 1.0)
            dsb = rcpool.tile([1, 512], F32, tag="dn")
            nc.vector.tensor_copy(dsb[:], zpss[h01][64:65, :])
            rcp = rcpool.tile([1, 512], F32, tag="rc")
            nc.vector.reciprocal_approx_fast(rcp[:], dsb[:])
            bc = bcpool.tile([64, 512], F32, tag="bc")
            nc.gpsimd.partition_broadcast(bc[:], rcp[:])
            nc.vector.tensor_mul(
                zpj[64 * h01:64 * (h01 + 1), pr, :],
                zpss[h01][0:64, :], bc[:],
            )

    def outproj(j, ps_pool):
        """Output projection for pq tile j: fp8 DR over both pairs."""
        zpj = zp_tiles[j]
        for c4 in range(4):
            ck = 4 * j + c4
            osb = opool.tile([128, 1024], F16, tag="osb")
            pps = [ps_pool.tile([128, 512], F32, tag="aux",
                               name=f"op{ck}{mt}") for mt in range(2)]
            for pr in range(2):
                for mt in range(2):
                    nc.tensor.matmul(
                        pps[mt][:],
                        zpj[:, pr, 128 * c4:128 * (c4 + 1)],
                        wosb[:, pr, 512 * mt:512 * (mt + 1)],
                        start=(pr == 0), stop=(pr == 1),
                    )
            for mt in range(2):
                nc.vector.tensor_copy(
                    osb[:, 512 * mt:512 * (mt + 1)], pps[mt][:])
            nc.sync.dma_start(outp[128 * ck:128 * (ck + 1), :], osb[:])

    zp_tiles = {}
    for j in range(PT):
        zp_tiles[j] = zppool.tile([128, 2, 512], F16, tag="zp",
                                  name=f"zp{j}")

    with tc.tile_pool(name="ps_sps", bufs=2, space="PSUM") as ps_sps, \
         tc.tile_pool(name="ps_zps", bufs=2, space="PSUM") as ps_zps, \
         tc.tile_pool(name="ps_aux", bufs=2, space="PSUM") as ps_aux:
        # ---- pr0 projections ----
        for t in ("v", "q", "k"):
            for h in range(2):
                proj_group(t, 0, h, ps_aux)
        # ---- pr0 attention, pr1 projections interleaved ----
        attn_pair(0, 3, ps_sps, ps_zps)
        proj_group("v", 1, 0, ps_aux)
        proj_group("v", 1, 1, ps_aux)
        attn_pair(0, 2, ps_sps, ps_zps)
        proj_group("q", 1, 0, ps_aux)
        proj_group("q", 1, 1, ps_aux)
        attn_pair(0, 1, ps_sps, ps_zps)
        proj_group("k", 1, 0, ps_aux)
        proj_group("k", 1, 1, ps_aux)
        attn_pair(0, 0, ps_sps, ps_zps)
        # ---- pr1 attention with trailing output projection ----
        attn_pair(1, 3, ps_sps, ps_zps)
        outproj(3, ps_aux)
        attn_pair(1, 2, ps_sps, ps_zps)
        outproj(2, ps_aux)
        attn_pair(1, 1, ps_sps, ps_zps)
        outproj(1, ps_aux)
        attn_pair(1, 0, ps_sps, ps_zps)
        outproj(0, ps_aux)


def _build():
    if "v2" in _BUILT:
        return _BUILT["v2"]
    from contextlib import ExitStack

    nc = bacc.Bacc("TRN2", target_bir_lowering=False, debug=False)
    aps = {
        "x8": nc.dram_tensor("x8", [MKD, 128, 2, 2048], FP8,
                             kind="ExternalInput").ap(),
        "wq": nc.dram_tensor("wq", [NPAIRS, MKD, 128, 2, 128], FP8,
                             kind="ExternalInput").ap(),
        "wk": nc.dram_tensor("wk", [NPAIRS, MKD, 128, 2, 128], FP8,
                             kind="ExternalInput").ap(),
        "wv": nc.dram_tensor("wv", [NPAIRS, 8, 128, 128], BF16,
                             kind="ExternalInput").ap(),
        "xbf": nc.dram_tensor("xbf", [8, 128, 2048], BF16,
                              kind="ExternalInput").ap(),
        "wo": nc.dram_tensor("wo", [128, 2, 1024], F16,
                             kind="ExternalInput").ap(),
        "bcol": nc.dram_tensor("bcol", [128, 6], F32,
                               kind="ExternalInput").ap(),
        "eyebf": nc.dram_tensor("eyebf", [128, 128], BF16,
                                kind="ExternalInput").ap(),
        "maskneg": nc.dram_tensor("maskneg", [128, 256], BF16,
                                  kind="ExternalInput").ap(),
        "outp": nc.dram_tensor("outp", [P, M], F16,
                               kind="ExternalOutput").ap(),
    }
    with tile.TileContext(nc) as tc:
        with ExitStack() as ctx, nc.allow_low_precision(
            reason="fp8 softmax kernel; verified numerically vs reference"
        ):
            _emit(nc, tc, aps, ctx)
    nc.compile()
    _BUILT["v2"] = nc
    return nc


def _host_inputs(x, kq, kk, kv, ko, bq, bk, bv):
    eyebf = np.eye(128, dtype=np.float32).astype(NP_BF16)
    r = np.arange(128)
    m1 = np.where(r[None, :] >= r[:, None], MASKC, 0.0)  # block: mask c >= r
    maskneg = np.concatenate(
        [m1, np.full((128, 128), MASKC)], axis=1
    ).astype(NP_BF16)

    def dr_pack(mat):  # [1024, F] -> [MKD, 128, 2, F]
        F = mat.shape[1]
        return np.ascontiguousarray(
            mat.reshape(MKD, 2, 128, F).transpose(0, 2, 1, 3)
        )

    in_maps = []
    for c in range(NCORES):
        b, k4 = divmod(c, 4)
        heads = [4 * k4 + i for i in range(HPC)]
        xdr = dr_pack(x[b].T).astype(NP_FP8)  # [4, 128, 2, 2048]
        xbf = np.ascontiguousarray(
            x[b].T.reshape(8, 128, 2048)
        ).astype(NP_BF16)

        def pairw(kern):
            out = np.empty((NPAIRS, MKD, 128, 2, 128), NP_FP8)
            for pr in range(NPAIRS):
                pairm = np.concatenate(
                    [kern[heads[2 * pr]], kern[heads[2 * pr + 1]]], axis=1
                )  # [1024, 128]
                out[pr] = dr_pack(pairm).astype(NP_FP8)
            return out

        def pairw16(kern):
            out = np.empty((NPAIRS, 8, 128, 128), NP_BF16)
            for pr in range(NPAIRS):
                pairm = np.concatenate(
                    [kern[heads[2 * pr]], kern[heads[2 * pr + 1]]], axis=1
                )  # [1024, 128]
                out[pr] = pairm.reshape(8, 128, 128).astype(NP_BF16)
            return out

        wo = np.stack(
            [np.concatenate([ko[heads[0]], ko[heads[1]]], axis=0),
             np.concatenate([ko[heads[2]], ko[heads[3]]], axis=0)], axis=1
        ).astype(np.float16)  # [128, 2, 1024]

        bcol = np.zeros((128, 6), np.float32)
        for pr in range(NPAIRS):
            for idx, bias in ((0, bq), (1, bk), (2, bv)):
                bcol[:, idx + 3 * pr] = np.concatenate(
                    [bias[heads[2 * pr]], bias[heads[2 * pr + 1]]]
                )

        in_maps.append({
            "x8": xdr, "xbf": xbf,
            "wq": pairw(kq), "wk": pairw(kk), "wv": pairw16(kv),
            "wo": wo, "bcol": bcol,
            "eyebf": eyebf, "maskneg": maskneg,
        })
    return in_maps


def kernel(x, kernel_query, kernel_key, kernel_value, kernel_out,
           bias_query, bias_key, bias_value, bias_out, _trace=False):
    x = np.asarray(x, np.float32)
    kq = np.asarray(kernel_query, np.float32)
    kk = np.asarray(kernel_key, np.float32)
    kv = np.asarray(kernel_value, np.float32)
    ko = np.asarray(kernel_out, np.float32)
    bq = np.asarray(bias_query, np.float32)
    bk = np.asarray(bias_key, np.float32)
    bv = np.asarray(bias_value, np.float32)
    bo = np.asarray(bias_out, np.float32)

    nc = _build()
    in_maps = _host_inputs(x, kq, kk, kv, ko, bq, bk, bv)
    res = bass_utils.run_bass_kernel_spmd(
        nc, in_maps, core_ids=list(range(NCORES)), trace=_trace
    )
    out = np.zeros((B, P, M), np.float32)
    for c in range(NCORES):
        out[c // 4] += res.results[c]["outp"].astype(np.float32)
    out += bo[None, None, :]

    # patch fully-masked query row P-1: uniform attention = mean_k v
    for b in range(B):
        xbar = x[b].mean(axis=0, dtype=np.float64)  # [M]
        row = np.zeros(M, np.float64)
        for n in range(N):
            zrow = xbar @ kv[n].astype(np.float64) + bv[n].astype(np.float64)
            row += zrow @ ko[n].astype(np.float64)
        out[b, P - 1, :] = (row + bo.astype(np.float64)).astype(np.float32)

    if _trace:
        kernel._last_result = res
    return out
